# revision 21
# baseline (speedup 1.0000x reference)
"""Trainium2 Bass kernel for AttributionCentroidTracker.

Reference computation (B=512, V=32768, C=16):
    Wg[b, v]   = W_eff[b, v, labels[b]]
    attr[b, v] = |sparse_vector[b, v] * Wg[b, v]|
    sums[c, v] = segment_sum(attr, labels)       # [C, V]
    mean       = sums / max(counts, 1)
    out[c]     = centroids[c]                     if counts[c] == 0
               = mean[c]                          if not initialized[c]
               = M*centroids[c] + (1-M)*mean[c]   otherwise

Device strategy (8 cores, sharded along V — per-class sums are complete
locally per V-slice, so no cross-core reduction is needed):
  - b (512) lives on the 128 SBUF partitions in 4 groups of 128.
  - W streams as PLAIN f32 over HWDGE (measured 395 GB/s/core vs only
    310 GB/s for the SWDGE cast-DMA path — the in-DMA dtype conversion
    is the slower wall).  ScalarE then does |W| WITH the f32->bf16 cast
    in one Abs pass, writing the bf16 result over the front half of the
    same tile (writer address trails reader address, so in-place is
    safe).
  - VectorE multiplies by |sv| broadcast along c (stride-0, 1x mode).
  - Segment-sum on TensorE with plain one-hot lhsT [128,16] per batch
    group and contiguous rhs [128,512] (v32*c16 layout), accumulating
    over the 4 groups into 4 PSUM banks per tile.  psum[c', (v,c)]
    holds per-class sums of ALL 16 channels; only the diagonal c'==c is
    wanted.
  - Banks are evacuated scaled by a_c into a bf16 staging tile (split
    between ScalarE and VectorE); every TBATCH tiles the diagonal is
    pulled out with 16 accumulating selection matmuls E_cc^T @
    stage[:, :, :, c] and added into the f32 accumulator out_sb, which
    is pre-loaded with b_c*centroids (host-computed).
  - a/b host math: a = (init ? (1-M)/n : 1/n) if present else 0,
    b = (init ? M : 0) if present else 1.
"""

import os
import sys

import numpy as np

if "/opt/trn_rl_repo" not in sys.path:
    sys.path.insert(0, "/opt/trn_rl_repo")

B, V, C = 512, 32768, 16
NCORES = 8
VSH = V // NCORES            # 4096 columns of V per core
P = 128                      # SBUF partitions
BG = B // P                  # 4 batch groups
VC = 128                     # v-chunk per tile
NVC = VSH // VC              # 32 tiles per core
NSUB = 4                     # psum banks per tile (32 v each)
VSUB = VC // NSUB            # 32
TBATCH = 2                   # tiles per extraction batch
NQ = 4                       # sv quarter-loads
VQ = VSH // NQ               # 1024
STEPS_PER_EPOCH = 1000
MOMENTUM = 1.0 - 2.0 / (STEPS_PER_EPOCH + 1)

_CACHE = {}

last_exec_time_ns = None
last_results = None


def _build_nc():
    import concourse.bacc as bacc
    import concourse.tile as tile
    from concourse import mybir

    f32 = mybir.dt.float32
    bf16 = mybir.dt.bfloat16
    Copy = mybir.ActivationFunctionType.Copy
    Abs = mybir.ActivationFunctionType.Abs
    nc = bacc.Bacc("TRN2", target_bir_lowering=False, debug=False)

    w = nc.dram_tensor("w", [B, VSH, C], f32, kind="ExternalInput")
    sv = nc.dram_tensor("sv", [B, VSH], f32, kind="ExternalInput")
    oh = nc.dram_tensor("oh", [P, BG * C], bf16, kind="ExternalInput")
    sel = nc.dram_tensor("sel", [C, C * C], bf16, kind="ExternalInput")
    centb = nc.dram_tensor("centb", [C, VSH], f32, kind="ExternalInput")
    avec = nc.dram_tensor("avec", [C, 1], f32, kind="ExternalInput")
    out = nc.dram_tensor("out", [C, VSH], f32, kind="ExternalOutput")

    # b = g*128 + p  ->  partition p, group g
    w_r = w.ap().rearrange("(g p) v c -> p g v c", p=P)      # [128, 4, VSH, 16]
    sv_r = sv.ap().rearrange("(g p) v -> p g v", p=P)        # [128, 4, VSH]

    with tile.TileContext(nc) as tc:
        with (
            tc.tile_pool(name="const", bufs=1) as cpool,
            tc.tile_pool(name="wp", bufs=4) as wpool,
            tc.tile_pool(name="svq", bufs=2) as qpool,
            tc.tile_pool(name="stg", bufs=2) as spool,
            tc.tile_pool(name="psum", bufs=8, space="PSUM") as ppool,
        ):
            # |sv| quarters as bf16 via SWDGE cast-DMA (small; the big W
            # stream stays on the faster plain-HWDGE path), abs in place.
            def issue_sv_quarter(q):
                qsl = slice(q * VQ, (q + 1) * VQ)
                svq = qpool.tile([P, BG * VQ], dtype=bf16, tag="svq")
                svq3 = svq[:].rearrange("p (g v) -> p g v", g=BG)
                nc.gpsimd.dma_start(out=svq3, in_=sv_r[:, :, qsl])
                qv = svq[:].bitcast(mybir.dt.int32)
                nc.vector.tensor_scalar(
                    out=qv,
                    in0=qv,
                    scalar1=0x7FFF7FFF,
                    scalar2=None,
                    op0=mybir.AluOpType.bitwise_and,
                )
                return svq3

            svq_cur = issue_sv_quarter(0)

            oh_sb = cpool.tile([P, BG * C], dtype=bf16)
            nc.sync.dma_start(out=oh_sb[:], in_=oh.ap())
            sel_sb = cpool.tile([C, C * C], dtype=bf16)
            nc.sync.dma_start(out=sel_sb[:], in_=sel.ap())
            avec_sb = cpool.tile([C, 1], dtype=f32)
            nc.sync.dma_start(out=avec_sb[:], in_=avec.ap())

            # accumulator pre-loaded with b_c * centroids
            out_sb = cpool.tile([C, VSH], dtype=f32)
            nc.sync.dma_start(out=out_sb[:], in_=centb.ap())

            def issue_w_dma(i):
                vlo = i * VC
                wt = wpool.tile([P, BG * VC * C], dtype=f32, tag="wt")
                wt4 = wt[:].rearrange("p (g v c) -> p g v c", g=BG, v=VC)
                nc.sync.dma_start(out=wt4, in_=w_r[:, :, vlo : vlo + VC, :])
                return wt

            PREFETCH = 3
            prefetched = {}
            for i in range(min(PREFETCH, NVC)):
                prefetched[i] = issue_w_dma(i)

            nsv = 1
            stage = None
            svq_next = None
            for i in range(NVC):
                vlo = i * VC
                ib = i % TBATCH

                if i + PREFETCH < NVC:
                    prefetched[i + PREFETCH] = issue_w_dma(i + PREFETCH)
                # next sv quarter two tiles ahead of first use
                if nsv < NQ and i == (nsv * NVC // NQ) - 2:
                    svq_next = issue_sv_quarter(nsv)
                if nsv < NQ and i == (nsv * NVC // NQ):
                    svq_cur = svq_next
                    nsv += 1

                wt = prefetched.pop(i)
                # |W| with f32->bf16 cast in one ScalarE pass, writing the
                # bf16 result over the tile's front half (in-place safe:
                # write addr 2k trails read addr 4k).
                wb = wt[:].bitcast(bf16)[:, 0 : BG * VC * C]
                nc.scalar.activation(wb, wt[:], Abs)
                wb4 = wb.rearrange("p (g v c) -> p g v c", g=BG, v=VC)

                # Y = |W| * |sv|  (|sv| broadcast along c, DVE 1x)
                vq = vlo - (vlo // VQ) * VQ
                in1 = (
                    svq_cur[:, :, vq : vq + VC]
                    .unsqueeze(3)
                    .broadcast_to([P, BG, VC, C])
                )
                nc.vector.tensor_tensor(
                    out=wb4, in0=wb4, in1=in1, op=mybir.AluOpType.mult
                )

                # segment-sum: ps[c', (v32, c)] += oh_g^T @ Y_g
                # (s outer / g inner so bank s finishes early and its
                # evacuation overlaps the remaining banks' matmuls)
                if ib == 0:
                    stage = spool.tile(
                        [C, TBATCH * VC * C], dtype=bf16, tag="stage"
                    )
                for s in range(NSUB):
                    ps = ppool.tile(
                        [C, VSUB * C],
                        dtype=mybir.dt.float32,
                        tag="ps",
                        name=f"ps{s}_{i}",
                    )
                    for g in range(BG):
                        off = g * (VC * C) + s * (VSUB * C)
                        nc.tensor.matmul(
                            out=ps[:],
                            lhsT=oh_sb[:, g * C : (g + 1) * C],
                            rhs=wb[:, off : off + VSUB * C],
                            start=(g == 0),
                            stop=(g == BG - 1),
                        )
                    # evacuate scaled by a_c into the bf16 staging tile,
                    # alternating engines to balance ACT/DVE load
                    soff = (ib * NSUB + s) * (VSUB * C)
                    dst = stage[:, soff : soff + VSUB * C]
                    if s % 2 == 0:
                        nc.scalar.activation(
                            dst, ps[:], Copy, bias=0.0, scale=avec_sb[:]
                        )
                    else:
                        nc.vector.tensor_scalar(
                            out=dst,
                            in0=ps[:],
                            scalar1=avec_sb[:],
                            scalar2=None,
                            op0=mybir.AluOpType.mult,
                        )

                # extraction batch: diagonal (c', (v,c)) c'==c via 16
                # accumulating selection matmuls E_cc^T @ stage[:, :, :, c]
                if ib == TBATCH - 1:
                    nchunk = TBATCH * NSUB
                    ps2 = ppool.tile(
                        [C, TBATCH * VC],
                        dtype=mybir.dt.float32,
                        tag="ps",
                        name=f"ps_diag_{i}",
                    )
                    stg4 = stage[:].rearrange(
                        "q (k v c) -> q k v c", k=nchunk, v=VSUB
                    )
                    for c in range(C):
                        nc.tensor.matmul(
                            out=ps2[:],
                            lhsT=sel_sb[:, c * C : (c + 1) * C],
                            rhs=stg4[:, :, :, c],
                            start=(c == 0),
                            stop=(c == C - 1),
                        )
                    ooff = (i - (TBATCH - 1)) * VC
                    nc.vector.tensor_tensor(
                        out=out_sb[:, ooff : ooff + TBATCH * VC],
                        in0=out_sb[:, ooff : ooff + TBATCH * VC],
                        in1=ps2[:],
                        op=mybir.AluOpType.add,
                    )

            nc.sync.dma_start(out=out.ap(), in_=out_sb[:])

    nc.finalize()
    return nc


def _get_nc():
    if "nc" not in _CACHE:
        _CACHE["nc"] = _build_nc()
    return _CACHE["nc"]


def kernel(sparse_vector, W_eff, labels, centroids, initialized):
    global last_exec_time_ns, last_results
    import ml_dtypes
    from concourse.bass_utils import run_bass_kernel_spmd

    sv = np.ascontiguousarray(np.asarray(sparse_vector, dtype=np.float32))
    w = np.asarray(W_eff, dtype=np.float32)
    lab = np.asarray(labels).astype(np.int64)
    cent = np.asarray(centroids, dtype=np.float32)
    init = np.asarray(initialized).astype(bool)

    # Host-side label-derived constants (tiny) — keep the program generic.
    ohm = lab[:, None] == np.arange(C)[None, :]          # [B, C] bool
    counts = ohm.sum(axis=0).astype(np.float64)          # [C]
    present = counts > 0
    safe = np.maximum(counts, 1.0)
    a = np.where(present, np.where(init, (1.0 - MOMENTUM) / safe, 1.0 / safe), 0.0)
    b = np.where(present, np.where(init, MOMENTUM, 0.0), 1.0)
    avec = a.astype(np.float32).reshape(C, 1)
    centb = (b[:, None] * cent.astype(np.float64)).astype(np.float32)  # [C, V]

    # Plain one-hot lhsT blocks: oh[p, g*C + c] = 1 iff labels[g*128+p]==c
    lab2 = lab.reshape(BG, P)                            # [g, p]
    oh = np.zeros((P, BG * C), np.float32)
    for g in range(BG):
        oh[np.arange(P), g * C + lab2[g]] = 1.0
    oh = oh.astype(ml_dtypes.bfloat16)

    # Diagonal-selection lhsT blocks: sel[p, c*C+m] = 1 iff p==c==m
    selm = np.zeros((C, C * C), np.float32)
    for c in range(C):
        selm[c, c * C + c] = 1.0
    selm = selm.astype(ml_dtypes.bfloat16)

    nc = _get_nc()
    in_maps = []
    for i in range(NCORES):
        s = i * VSH
        in_maps.append(
            {
                "w": np.ascontiguousarray(w[:, s : s + VSH, :]),
                "sv": np.ascontiguousarray(sv[:, s : s + VSH]),
                "oh": oh,
                "sel": selm,
                "centb": np.ascontiguousarray(centb[:, s : s + VSH]),
                "avec": avec,
            }
        )

    res = run_bass_kernel_spmd(nc, in_maps, core_ids=list(range(NCORES)))
    last_exec_time_ns = res.exec_time_ns
    last_results = res
    return np.concatenate([res.results[i]["out"] for i in range(NCORES)], axis=1)


# revision 23
# speedup vs baseline: 1.0038x; 1.0038x over previous
"""Trainium2 Bass kernel for AttributionCentroidTracker.

Reference computation (B=512, V=32768, C=16):
    Wg[b, v]   = W_eff[b, v, labels[b]]
    attr[b, v] = |sparse_vector[b, v] * Wg[b, v]|
    sums[c, v] = segment_sum(attr, labels)       # [C, V]
    mean       = sums / max(counts, 1)
    out[c]     = centroids[c]                     if counts[c] == 0
               = mean[c]                          if not initialized[c]
               = M*centroids[c] + (1-M)*mean[c]   otherwise

Device strategy (8 cores, sharded along V — per-class sums are complete
locally per V-slice, so no cross-core reduction is needed):
  - b (512) lives on the 128 SBUF partitions in 4 groups of 128.
  - W streams as PLAIN f32 over HWDGE (measured 395 GB/s/core vs only
    310 GB/s for the SWDGE cast-DMA path — the in-DMA dtype conversion
    is the slower wall).  ScalarE then does |W| WITH the f32->bf16 cast
    in one Abs pass, writing the bf16 result over the front half of the
    same tile (writer address trails reader address, so in-place is
    safe).
  - VectorE multiplies by |sv| broadcast along c (stride-0, 1x mode).
  - Segment-sum on TensorE with plain one-hot lhsT [128,16] per batch
    group and contiguous rhs [128,512] (v32*c16 layout), accumulating
    over the 4 groups into 4 PSUM banks per tile.  psum[c', (v,c)]
    holds per-class sums of ALL 16 channels; only the diagonal c'==c is
    wanted.
  - Banks are evacuated scaled by a_c into a bf16 staging tile (split
    between ScalarE and VectorE); every TBATCH tiles the diagonal is
    pulled out with 16 accumulating selection matmuls E_cc^T @
    stage[:, :, :, c] and added into the f32 accumulator out_sb, which
    is pre-loaded with b_c*centroids (host-computed).
  - a/b host math: a = (init ? (1-M)/n : 1/n) if present else 0,
    b = (init ? M : 0) if present else 1.
"""

import os
import sys

import numpy as np

if "/opt/trn_rl_repo" not in sys.path:
    sys.path.insert(0, "/opt/trn_rl_repo")

B, V, C = 512, 32768, 16
NCORES = 8
VSH = V // NCORES            # 4096 columns of V per core
P = 128                      # SBUF partitions
BG = B // P                  # 4 batch groups
VC = 128                     # v-chunk per tile
NVC = VSH // VC              # 32 tiles per core
NSUB = 4                     # psum banks per tile (32 v each)
VSUB = VC // NSUB            # 32
TBATCH = 4                   # tiles per extraction batch
NQ = 8                       # sv chunk-loads (eighths)
VQ = VSH // NQ               # 512
STEPS_PER_EPOCH = 1000
MOMENTUM = 1.0 - 2.0 / (STEPS_PER_EPOCH + 1)

_CACHE = {}

last_exec_time_ns = None
last_results = None


def _build_nc():
    import concourse.bacc as bacc
    import concourse.tile as tile
    from concourse import mybir

    f32 = mybir.dt.float32
    bf16 = mybir.dt.bfloat16
    Copy = mybir.ActivationFunctionType.Copy
    Abs = mybir.ActivationFunctionType.Abs
    nc = bacc.Bacc("TRN2", target_bir_lowering=False, debug=False)

    w = nc.dram_tensor("w", [B, VSH, C], f32, kind="ExternalInput")
    sv = nc.dram_tensor("sv", [B, VSH], f32, kind="ExternalInput")
    oh = nc.dram_tensor("oh", [P, BG * C], bf16, kind="ExternalInput")
    sel = nc.dram_tensor("sel", [C, C * C], bf16, kind="ExternalInput")
    centb = nc.dram_tensor("centb", [C, VSH], f32, kind="ExternalInput")
    avec = nc.dram_tensor("avec", [C, 1], f32, kind="ExternalInput")
    out = nc.dram_tensor("out", [C, VSH], f32, kind="ExternalOutput")

    # b = g*128 + p  ->  partition p, group g
    w_r = w.ap().rearrange("(g p) v c -> p g v c", p=P)      # [128, 4, VSH, 16]
    sv_r = sv.ap().rearrange("(g p) v -> p g v", p=P)        # [128, 4, VSH]

    with tile.TileContext(nc) as tc:
        with (
            tc.tile_pool(name="const", bufs=1) as cpool,
            tc.tile_pool(name="wp", bufs=4) as wpool,
            tc.tile_pool(name="svq", bufs=2) as qpool,
            tc.tile_pool(name="stg", bufs=2) as spool,
            tc.tile_pool(name="psum", bufs=8, space="PSUM") as ppool,
        ):
            # |sv| quarters as bf16 via SWDGE cast-DMA (small; the big W
            # stream stays on the faster plain-HWDGE path), abs in place.
            def issue_sv_quarter(q):
                qsl = slice(q * VQ, (q + 1) * VQ)
                svq = qpool.tile([P, BG * VQ], dtype=bf16, tag="svq")
                svq3 = svq[:].rearrange("p (g v) -> p g v", g=BG)
                nc.gpsimd.dma_start(out=svq3, in_=sv_r[:, :, qsl])
                qv = svq[:].bitcast(mybir.dt.int32)
                nc.vector.tensor_scalar(
                    out=qv,
                    in0=qv,
                    scalar1=0x7FFF7FFF,
                    scalar2=None,
                    op0=mybir.AluOpType.bitwise_and,
                )
                return svq3

            svq_cur = issue_sv_quarter(0)

            oh_sb = cpool.tile([P, BG * C], dtype=bf16)
            nc.sync.dma_start(out=oh_sb[:], in_=oh.ap())
            sel_sb = cpool.tile([C, C * C], dtype=bf16)
            nc.sync.dma_start(out=sel_sb[:], in_=sel.ap())
            avec_sb = cpool.tile([C, 1], dtype=f32)
            nc.sync.dma_start(out=avec_sb[:], in_=avec.ap())

            # accumulator pre-loaded with b_c * centroids
            out_sb = cpool.tile([C, VSH], dtype=f32)
            nc.sync.dma_start(out=out_sb[:], in_=centb.ap())

            def issue_w_dma(i):
                vlo = i * VC
                wt = wpool.tile([P, BG * VC * C], dtype=f32, tag="wt")
                wt4 = wt[:].rearrange("p (g v c) -> p g v c", g=BG, v=VC)
                nc.sync.dma_start(out=wt4, in_=w_r[:, :, vlo : vlo + VC, :])
                return wt

            PREFETCH = 3
            prefetched = {}
            for i in range(min(PREFETCH, NVC)):
                prefetched[i] = issue_w_dma(i)

            nsv = 1
            stage = None
            svq_next = None
            for i in range(NVC):
                vlo = i * VC
                ib = i % TBATCH

                if i + PREFETCH < NVC:
                    prefetched[i + PREFETCH] = issue_w_dma(i + PREFETCH)
                # next sv quarter two tiles ahead of first use
                if nsv < NQ and i == (nsv * NVC // NQ) - 2:
                    svq_next = issue_sv_quarter(nsv)
                if nsv < NQ and i == (nsv * NVC // NQ):
                    svq_cur = svq_next
                    nsv += 1

                wt = prefetched.pop(i)
                wb = wt[:].bitcast(bf16)[:, 0 : BG * VC * C]
                HALF = BG * VC * C // 2
                vq = vlo - (vlo // VQ) * VQ
                if ib == 0:
                    stage = spool.tile(
                        [C, TBATCH * VC * C], dtype=bf16, tag="stage"
                    )
                # stage layout is (c, k, v) so the diagonal matmuls below
                # read contiguous [16, 512] rhs slices per class
                nchunk = TBATCH * NSUB
                stg_ev = stage[:].rearrange(
                    "q (c k v) -> q k v c", c=C, k=nchunk, v=VSUB
                )

                for h in range(2):
                    # |W| with f32->bf16 cast in one ScalarE pass, writing
                    # the bf16 result over the tile's front bytes (safe in
                    # place: write addr 2k trails read addr 4k, per half)
                    hsl = slice(h * HALF, (h + 1) * HALF)
                    nc.scalar.activation(wb[:, hsl], wt[:, hsl], Abs)
                    # Y = |W| * |sv|  (|sv| broadcast along c, DVE 1x)
                    wb4h = wb[:, hsl].rearrange(
                        "p (g v c) -> p g v c", g=BG // 2, v=VC
                    )
                    in1 = (
                        svq_cur[:, 2 * h : 2 * h + 2, vq : vq + VC]
                        .unsqueeze(3)
                        .broadcast_to([P, BG // 2, VC, C])
                    )
                    nc.vector.tensor_tensor(
                        out=wb4h, in0=wb4h, in1=in1, op=mybir.AluOpType.mult
                    )

                # segment-sum: ps[c', (v32, c)] += oh_g^T @ Y_g
                # (g outer so each one-hot block is loaded once into PE)
                pss = []
                for s in range(NSUB):
                    pss.append(
                        ppool.tile(
                            [C, VSUB * C],
                            dtype=mybir.dt.float32,
                            tag="ps",
                            name=f"ps{s}_{i}",
                        )
                    )
                for g in range(BG):
                    for s in range(NSUB):
                        off = g * (VC * C) + s * (VSUB * C)
                        nc.tensor.matmul(
                            out=pss[s][:],
                            lhsT=oh_sb[:, g * C : (g + 1) * C],
                            rhs=wb[:, off : off + VSUB * C],
                            start=(g == 0),
                            stop=(g == BG - 1),
                        )
                for s in range(NSUB):
                    # evacuate scaled by a_c into the staging tile in
                    # (c, k, v) order, alternating engines for balance
                    k = ib * NSUB + s
                    dst = stg_ev[:, k]
                    if s % 2 == 0:
                        nc.scalar.activation(
                            dst, pss[s][:], Copy, bias=0.0, scale=avec_sb[:]
                        )
                    else:
                        nc.vector.tensor_scalar(
                            out=dst,
                            in0=pss[s][:],
                            scalar1=avec_sb[:],
                            scalar2=None,
                            op0=mybir.AluOpType.mult,
                        )

                # extraction batch: diagonal via 16 accumulating selection
                # matmuls E_cc^T @ stage[:, c-block] (contiguous rhs)
                if ib == TBATCH - 1:
                    ps2 = ppool.tile(
                        [C, TBATCH * VC],
                        dtype=mybir.dt.float32,
                        tag="ps",
                        name=f"ps_diag_{i}",
                    )
                    for c in range(C):
                        nc.tensor.matmul(
                            out=ps2[:],
                            lhsT=sel_sb[:, c * C : (c + 1) * C],
                            rhs=stage[:, c * nchunk * VSUB : (c + 1) * nchunk * VSUB],
                            start=(c == 0),
                            stop=(c == C - 1),
                        )
                    ooff = (i - (TBATCH - 1)) * VC
                    nc.vector.tensor_tensor(
                        out=out_sb[:, ooff : ooff + TBATCH * VC],
                        in0=out_sb[:, ooff : ooff + TBATCH * VC],
                        in1=ps2[:],
                        op=mybir.AluOpType.add,
                    )

            nc.sync.dma_start(out=out.ap(), in_=out_sb[:])

    nc.finalize()
    return nc


def _get_nc():
    if "nc" not in _CACHE:
        _CACHE["nc"] = _build_nc()
    return _CACHE["nc"]


def kernel(sparse_vector, W_eff, labels, centroids, initialized):
    global last_exec_time_ns, last_results
    import ml_dtypes
    from concourse.bass_utils import run_bass_kernel_spmd

    sv = np.ascontiguousarray(np.asarray(sparse_vector, dtype=np.float32))
    w = np.asarray(W_eff, dtype=np.float32)
    lab = np.asarray(labels).astype(np.int64)
    cent = np.asarray(centroids, dtype=np.float32)
    init = np.asarray(initialized).astype(bool)

    # Host-side label-derived constants (tiny) — keep the program generic.
    ohm = lab[:, None] == np.arange(C)[None, :]          # [B, C] bool
    counts = ohm.sum(axis=0).astype(np.float64)          # [C]
    present = counts > 0
    safe = np.maximum(counts, 1.0)
    a = np.where(present, np.where(init, (1.0 - MOMENTUM) / safe, 1.0 / safe), 0.0)
    b = np.where(present, np.where(init, MOMENTUM, 0.0), 1.0)
    avec = a.astype(np.float32).reshape(C, 1)
    centb = (b[:, None] * cent.astype(np.float64)).astype(np.float32)  # [C, V]

    # Plain one-hot lhsT blocks: oh[p, g*C + c] = 1 iff labels[g*128+p]==c
    lab2 = lab.reshape(BG, P)                            # [g, p]
    oh = np.zeros((P, BG * C), np.float32)
    for g in range(BG):
        oh[np.arange(P), g * C + lab2[g]] = 1.0
    oh = oh.astype(ml_dtypes.bfloat16)

    # Diagonal-selection lhsT blocks: sel[p, c*C+m] = 1 iff p==c==m
    selm = np.zeros((C, C * C), np.float32)
    for c in range(C):
        selm[c, c * C + c] = 1.0
    selm = selm.astype(ml_dtypes.bfloat16)

    nc = _get_nc()
    in_maps = []
    for i in range(NCORES):
        s = i * VSH
        in_maps.append(
            {
                "w": np.ascontiguousarray(w[:, s : s + VSH, :]),
                "sv": np.ascontiguousarray(sv[:, s : s + VSH]),
                "oh": oh,
                "sel": selm,
                "centb": np.ascontiguousarray(centb[:, s : s + VSH]),
                "avec": avec,
            }
        )

    res = run_bass_kernel_spmd(nc, in_maps, core_ids=list(range(NCORES)))
    last_exec_time_ns = res.exec_time_ns
    last_results = res
    return np.concatenate([res.results[i]["out"] for i in range(NCORES)], axis=1)
